# revision 9
# baseline (speedup 1.0000x reference)
"""BigBird-style block-sparse attention on 8 Trainium2 NeuronCores.

Problem: B=2, H=12, S=4096, D=64, BLK=64 (64 blocks), R=3 random blocks.
All mask inputs are ones (per the generator spec); rand_attn drives the
gather structure and is read host-side (the program itself is static).

Sharding: 24 (b,h) pairs -> 3 per core (data + head parallel).

Per-pair device algorithm, "ST" layout (keys on partitions, queries on
the free axis).  Middle query block l (1..62) attends, garbage-free:
  - W01: key pair {2p, 2p+1} shared by the query duo (2p, 2p+1)  [kt]
  - r01: host-gathered [rand0 | rand1] pair                      [k3 ]
  - wh:  window half key, 64-row strip at partition half wh%2    [kt]
  - r2:  rand2 strip at the other half (host-gathered)           [k3 ]
  - G:   global pack {0,63} (l=1 / l=62 use 64-row edge strips)
wh/r2 share one 64-col PSUM region via tile_position partition halves;
wh's PV reads v directly out of the natural parity half of the vn
chunk, r2's PV uses a 2-up packed gather (odd l upper, even l lower).
Blocks l = 0, 63 attend densely to all keys.  QK -> one exp per group
(scale fused) -> PV with a ones-column on V accumulating the softmax
denominator in ctx row 64.  Host divides and transposes.

DMA: per-queue throughput is descriptor-gen-bound (~131 ns per SBUF
partition row), so inputs are merged into TWO wide tensors: k64 (all
64-row data: q^T, k^T, packs, key gathers; 40 KB rows) and v128 (all
128-row V data with ones columns).  Output is staged in SBUF and
shipped as bf16 ctx + f32 denominator row.
"""

import numpy as np

B, H, S, D = 2, 12, 4096, 64
BLK = 64
NB = S // BLK            # 64
NPAIR = B * H            # 24
NCORE = 8
PPC = NPAIR // NCORE     # 3 pairs per core
NMID = 62                # l = 1..62
NT = 31                  # r2 duo chunks
SCALE = 0.125            # 1/sqrt(64)

# k64 column layout
K_QT = 0
K_KT = S
K_KTG = 2 * S
K_QTD = 2 * S + 128
K_VGE = 2 * S + 256
K_G3 = 2 * S + 256 + 130          # key gathers: per l, [r0|r1|wh|r2] (256)
K_W = K_G3 + NMID * 256           # 24450

# v128 column layout (65-wide chunks)
V_VN = 0                          # 33 chunks: 32 v-chunks + global pack
V_VR = 33 * 65                    # 62 chunk-pairs: [v_r0;v_r1|1][v_wh;v_r2|1]
V_W = (33 + 124) * 65             # 10205

_COMPILED = {}


def _build_host_arrays(query_layer, key_layer, value_layer, rand_attn):
    import ml_dtypes
    bf16 = ml_dtypes.bfloat16

    q = np.ascontiguousarray(query_layer, dtype=np.float32).reshape(NPAIR, S, D)
    k = np.ascontiguousarray(key_layer, dtype=np.float32).reshape(NPAIR, S, D)
    v = np.ascontiguousarray(value_layer, dtype=np.float32).reshape(NPAIR, S, D)
    r = np.ascontiguousarray(rand_attn, dtype=np.int64).reshape(NPAIR, NMID, 3)

    qt = q.transpose(0, 2, 1)                                # [24, 64, S]
    kt = k.transpose(0, 2, 1)

    kb = k.reshape(NPAIR, NB, BLK, D)
    vb = v.reshape(NPAIR, NB, BLK, D)
    bh = np.arange(NPAIR)[:, None, None]

    # ---- k64 ----
    ktg = np.concatenate([kb[:, 0], kb[:, NB - 1]], axis=1).transpose(0, 2, 1)
    qb = q.reshape(NPAIR, NB, BLK, D)
    qtd = np.concatenate([qb[:, 0], qb[:, NB - 1]], axis=1).transpose(0, 2, 1)
    one = np.ones((NPAIR, BLK, 1), np.float32)
    v63 = np.concatenate([vb[:, NB - 1], one], axis=2)       # [24, 64, 65]
    v0 = np.concatenate([vb[:, 0], one], axis=2)
    vge = np.concatenate([v63, v0], axis=2)                  # [24, 64, 130]
    ls_ = np.arange(1, NMID + 1)
    whb = np.where(ls_ % 2 == 1, ls_ + 1, ls_ - 1)
    whb = np.broadcast_to(whb[None, :], (NPAIR, NMID))
    i4 = np.concatenate([r[:, :, 0:2], whb[:, :, None], r[:, :, 2:3]], axis=2)
    gk = kb[bh, i4]                                          # [24, 62, 4, 64, 64]
    g3 = gk.transpose(0, 4, 1, 2, 3).reshape(NPAIR, D, NMID * 256)
    k64 = np.ascontiguousarray(
        np.concatenate([qt, kt, ktg, qtd, vge, g3], axis=2)
    ).astype(bf16)                                           # [24, 64, K_W]

    # ---- v128 ----
    vch = v.reshape(NPAIR, NB // 2, 128, D)
    o32 = np.ones((NPAIR, NB // 2, 128, 1), np.float32)
    vn = np.concatenate([vch, o32], axis=3)                  # [24, 32, 128, 65]
    gvg = np.concatenate([vb[:, 0], vb[:, NB - 1]], axis=1)  # [24, 128, 64]
    vg = np.concatenate([gvg, np.ones((NPAIR, 128, 1), np.float32)],
                        axis=2)[:, None]                     # [24, 1, 128, 65]
    vne = np.concatenate([vn, vg], axis=1)                   # [24, 33, 128, 65]

    gv01 = vb[bh, r[:, :, 0:2]].reshape(NPAIR, NMID, 128, D)
    o62 = np.ones((NPAIR, NMID, 128, 1), np.float32)
    vrr = np.concatenate([gv01, o62], axis=3)                # [24, 62, 128, 65]

    gvm = vb[bh, i4[:, :, 2:4]].reshape(NPAIR, NMID, 128, D)
    vmm = np.concatenate([gvm, o62], axis=3)                 # [24, 62, 128, 65]
    # interleave per l: [vrr_l | vmm_l] -> [24, 62, 2, 128, 65]
    vri = np.stack([vrr, vmm], axis=2).reshape(NPAIR, NMID * 2, 128, 65)

    v128 = np.ascontiguousarray(
        np.concatenate([vne, vri], axis=1).transpose(0, 2, 1, 3)
        .reshape(NPAIR, 128, V_W)
    ).astype(bf16)

    return dict(k64=k64, v128=v128)


def _fixup_multiwait(nc, mybir):
    """Split >1-sem-wait instructions (the Tile exit drain) into single-wait
    NoOps: this walrus build's CTRL codegen has one wait slot."""
    for fn in nc.m.functions:
        for bb in fn.blocks:
            insts = list(bb.instructions)
            out = []
            for inst in insts:
                si = inst.sync_info
                if si is not None and len(si.on_wait) > 1:
                    waits = list(si.on_wait)
                    for kk, w in enumerate(waits[:-1]):
                        nop = mybir.InstNoOp(
                            name=f"{inst.name}-wsplit{kk}",
                            opcode="NoOp",
                            engine=inst.engine,
                            sync_info=mybir.SyncInfo(on_wait=[w], on_update=[]),
                        )
                        out.append(nop)
                    si.on_wait = [waits[-1]]
                    inst.sync_info = si
                out.append(inst)
            bb.instructions = out


def _group_plan():
    """11 groups covering l=1..62.

    qk job: (dst_off, width, src, ctx_off, mrows, obase)
      out = st[obase:obase+mrows, off:off+w]
      src: ('kt',col,w) ('ktg',off,w) ('k3r',i) ('k3x',i) ('kts',block)
    pv job: (pt_off, width, src, ctx_off, krows, rbase)
      rhs = pt[rbase:rbase+krows, off:off+w]
      src: ('vn',chunk) ('vg',) ('vge',which) ('vr',i) ('vnh',block)
           ('vr2',t)
    """
    groups = []

    def build(ls_, singles, duos, g_edges):
        base_l = ls_[0]
        qk, pv = [], []
        off = 0
        g_ls = [l for l in ls_ if l not in g_edges]
        assert g_ls == list(range(g_ls[0], g_ls[0] + len(g_ls)))
        w = len(g_ls) * BLK
        qk.append((off, w, ('ktg', 0, 128), (g_ls[0] - base_l) * BLK, 128, 0))
        pv.append((off, w, ('vg',), (g_ls[0] - base_l) * BLK, 128, 0))
        off += w
        for l in g_edges:
            ko, vw = ((64, 0) if l == 1 else (0, 1))
            qk.append((off, 64, ('ktg', ko, 64), (l - base_l) * BLK, 64, 0))
            pv.append((off, 64, ('vge', vw), (l - base_l) * BLK, 64, 0))
            off += 64
        for l in singles:
            p = l // 2 if l % 2 == 0 else (l - 1) // 2
            qk.append((off, 64, ('kt', p * 128, 128), (l - base_l) * BLK,
                       128, 0))
            pv.append((off, 64, ('vn', p), (l - base_l) * BLK, 128, 0))
            off += 64
        for le in duos:
            p = le // 2
            qk.append((off, 128, ('kt', p * 128, 128), (le - base_l) * BLK,
                       128, 0))
            pv.append((off, 128, ('vn', p), (le - base_l) * BLK, 128, 0))
            off += 128
        for l in ls_:
            i = l - 1
            qk.append((off, 64, ('k3r', i), (l - base_l) * BLK, 128, 0))
            pv.append((off, 64, ('vr', i), (l - base_l) * BLK, 128, 0))
            off += 64
        for l in ls_:
            i = l - 1
            c = (l - base_l) * BLK
            qk.append((off, 64, ('k3m', i), c, 128, 0))
            pv.append((off, 64, ('vm', i), c, 128, 0))
            off += 64
        for o_, w_, _s, _c, _m, _b in qk:
            assert o_ // 512 == (o_ + w_ - 1) // 512, (o_, w_)
        assert off <= 1536
        return dict(ls=ls_, qk=qk, pv=pv, used=off)

    groups.append(build([1, 2, 3, 4, 5], singles=[1], duos=[2, 4],
                        g_edges=[1]))
    for kk in range(1, 10):
        a = 6 * kk
        groups.append(build(list(range(a, a + 6)), singles=[],
                            duos=[a, a + 2, a + 4], g_edges=[]))
    groups.append(build([60, 61, 62], singles=[62], duos=[60], g_edges=[62]))

    assert [l for g in groups for l in g['ls']] == list(range(1, 63))
    return groups


GROUPS = _group_plan()


def _build_program(apply_fixup=True):
    import sys
    if "/opt/trn_rl_repo" not in sys.path:
        sys.path.insert(0, "/opt/trn_rl_repo")
    import concourse.bass as bass
    import concourse.mybir as mybir
    from concourse.tile import TileContext

    f32 = mybir.dt.float32
    bf16 = mybir.dt.bfloat16
    EXP = mybir.ActivationFunctionType.Exp

    nc = bass.Bass("TRN2", target_bir_lowering=False, debug=False,
                   num_devices=NCORE)

    d_k64 = nc.dram_tensor("k64", [PPC, D, K_W], bf16,
                           kind="ExternalInput").ap()
    d_v128 = nc.dram_tensor("v128", [PPC, 128, V_W], bf16,
                            kind="ExternalInput").ap()
    d_ctx = nc.dram_tensor("ctx", [PPC, 64, S], bf16,
                           kind="ExternalOutput").ap()
    d_den = nc.dram_tensor("den", [PPC, 1, S], f32, kind="ExternalOutput").ap()

    DW = [(0, 12), (12, 12), (24, 8)]   # dense waves

    with TileContext(nc) as tc:
        with tc.tile_pool(name="sb", bufs=2) as sb, \
             tc.tile_pool(name="ps", bufs=2, space="PSUM") as ps, \
             tc.tile_pool(name="ptp", bufs=4) as ptp, \
             tc.tile_pool(name="aux", bufs=2) as aux:

            for p in range(PPC):
                k64 = sb.tile([D, K_W], bf16, name=f"k64{p}", tag="k64")
                v128 = sb.tile([128, V_W], bf16, name=f"v128{p}", tag="v128")
                nc.sync.dma_start(out=k64, in_=d_k64[p])
                nc.scalar.dma_start(out=v128, in_=d_v128[p])

                qt = k64[:, K_QT:K_QT + S]
                kt = k64[:, K_KT:K_KT + S]

                def src_k(src):
                    kind = src[0]
                    if kind == 'kt':
                        return kt[:, src[1]:src[1] + src[2]]
                    if kind == 'ktg':
                        return k64[:, K_KTG + src[1]:K_KTG + src[1] + src[2]]
                    if kind == 'k3r':
                        a = K_G3 + src[1] * 256
                        return k64[:, a:a + 128]
                    if kind == 'k3m':
                        a = K_G3 + src[1] * 256 + 128
                        return k64[:, a:a + 128]
                    raise KeyError(src)

                def src_v(src):
                    kind = src[0]
                    if kind == 'vn':
                        return v128[:, src[1] * 65:(src[1] + 1) * 65]
                    if kind == 'vg':
                        return v128[:, 32 * 65:33 * 65]
                    if kind == 'vge':
                        return k64[:, K_VGE + src[1] * 65:
                                   K_VGE + (src[1] + 1) * 65]
                    if kind == 'vr':
                        a = V_VR + src[1] * 130
                        return v128[:, a:a + 65]
                    if kind == 'vm':
                        a = V_VR + src[1] * 130 + 65
                        return v128[:, a:a + 65]
                    raise KeyError(src)

                og = aux.tile([128, S], bf16, name=f"og{p}", tag="og")
                ogd = aux.tile([1, S], f32, name=f"ogd{p}", tag="ogd")

                # ---------------- dense blocks l = 0, 63 ----------------
                qtd = k64[:, K_QTD:K_QTD + 128]
                ctxd = ps.tile([128, 512], f32, name=f"ctxd{p}", tag="ctx",
                               bufs=2)
                for wv, (c0, nch) in enumerate(DW):
                    wd = nch * 128
                    std = ps.tile([128, 1536], f32, name=f"std{p}_{wv}",
                                  tag="st", bufs=2)
                    for cc in range(nch):
                        c = c0 + cc
                        nc.tensor.matmul(
                            std[:, cc * 128:(cc + 1) * 128],
                            lhsT=kt[:, c * 128:(c + 1) * 128],
                            rhs=qtd,
                            start=True, stop=True,
                        )
                    ptd = ptp.tile([128, 1536], bf16, name=f"ptd{p}_{wv}",
                                   tag="pt", bufs=4)
                    nc.scalar.activation(ptd[:, 0:wd], std[:, 0:wd], EXP,
                                         scale=SCALE)
                    for cc in range(nch):
                        c = c0 + cc
                        nc.tensor.matmul(
                            ctxd[0:65, 0:128],
                            lhsT=v128[:, c * 65:(c + 1) * 65],
                            rhs=ptd[:, cc * 128:(cc + 1) * 128],
                            start=(c == 0), stop=(c == 31),
                        )
                nc.vector.tensor_copy(og[0:64, 0:BLK], ctxd[0:64, 0:BLK])
                nc.vector.tensor_copy(og[0:64, S - BLK:S], ctxd[0:64, BLK:128])
                nc.vector.tensor_copy(ogd[0:1, 0:BLK], ctxd[64:65, 0:BLK])
                nc.vector.tensor_copy(ogd[0:1, S - BLK:S], ctxd[64:65, BLK:128])

                # ---------------- middle groups ----------------
                for g, plan in enumerate(GROUPS):
                    ls = plan['ls']
                    base_l = ls[0]
                    W = len(ls) * BLK
                    used = plan['used']

                    st = ps.tile([128, 1536], f32, name=f"st{p}_{g}", tag="st",
                                 bufs=2)
                    for off, w, src, _c, mrows, obase in plan['qk']:
                        nc.tensor.matmul(
                            st[obase:obase + mrows, off:off + w],
                            lhsT=src_k(src),
                            rhs=qt[:, (base_l * BLK) + _c:
                                   (base_l * BLK) + _c + w],
                            start=True, stop=True,
                        )
                    pt = ptp.tile([128, 1536], bf16, name=f"pt{p}_{g}",
                                  tag="pt", bufs=4)
                    nc.scalar.activation(pt[:, 0:used], st[:, 0:used], EXP,
                                         scale=SCALE)

                    ctx = ps.tile([128, 512], f32, name=f"ctx{p}_{g}",
                                  tag="ctx", bufs=2)
                    pv = plan['pv']
                    for idx, (off, w, src, c, krows, rbase) in enumerate(pv):
                        nc.tensor.matmul(
                            ctx[0:65, c:c + w],
                            lhsT=src_v(src),
                            rhs=pt[rbase:rbase + krows, off:off + w],
                            start=(idx == 0), stop=(idx == len(pv) - 1),
                        )

                    nc.vector.tensor_copy(
                        og[0:64, base_l * BLK: base_l * BLK + W],
                        ctx[0:64, 0:W])
                    nc.vector.tensor_copy(
                        ogd[0:1, base_l * BLK: base_l * BLK + W],
                        ctx[64:65, 0:W])
                    if ls[-1] == 35:
                        nc.gpsimd.dma_start(out=d_ctx[p][:, 0:S // 2],
                                            in_=og[0:64, 0:S // 2])
                nc.gpsimd.dma_start(out=d_ctx[p][:, S // 2:S],
                                    in_=og[0:64, S // 2:S])
                nc.gpsimd.dma_start(out=d_den[p], in_=ogd)

    if apply_fixup:
        _fixup_multiwait(nc, mybir)
    return nc


def _get_program():
    if "nc" not in _COMPILED:
        _COMPILED["nc"] = _build_program()
    return _COMPILED["nc"]


def kernel(query_layer, key_layer, value_layer, band_mask, from_mask, to_mask,
           from_blocked_mask, to_blocked_mask, rand_attn):
    import sys
    if "/opt/trn_rl_repo" not in sys.path:
        sys.path.insert(0, "/opt/trn_rl_repo")
    from concourse.bass_utils import run_bass_kernel_spmd

    arrs = _build_host_arrays(query_layer, key_layer, value_layer, rand_attn)
    nc = _get_program()

    in_maps = []
    for c in range(NCORE):
        sl = slice(c * PPC, (c + 1) * PPC)
        in_maps.append({k: np.ascontiguousarray(v[sl]) for k, v in arrs.items()})

    res = run_bass_kernel_spmd(nc, in_maps, list(range(NCORE)))

    ctx = np.stack([res.results[c]["ctx"] for c in range(NCORE)])
    den = np.stack([res.results[c]["den"] for c in range(NCORE)])
    ctx = ctx.reshape(NPAIR, 64, S).astype(np.float64)
    den = den.reshape(NPAIR, 1, S).astype(np.float64)
    ctx = ctx / den                                                  # [24,64,S]
    ctx = ctx.transpose(0, 2, 1).reshape(B, H, S, D)                 # [B,H,S,D]
    out = ctx.transpose(0, 2, 1, 3).astype(np.float32)               # [B,S,H,D]
    return np.ascontiguousarray(out)


# revision 10
# speedup vs baseline: 1.0942x; 1.0942x over previous
"""BigBird-style block-sparse attention on 8 Trainium2 NeuronCores.

Problem: B=2, H=12, S=4096, D=64, BLK=64 (64 blocks), R=3 random blocks.
All mask inputs are ones (per the generator spec); rand_attn drives the
gather structure and is read host-side (the program itself is static).

Sharding: 24 (b,h) pairs -> 3 per core (data + head parallel).

Per-pair device algorithm, "ST" layout (keys on partitions, queries on
the free axis).  Middle query block l (1..62) attends, garbage-free:
  - W01: key pair {2p, 2p+1} shared by the query duo (2p, 2p+1)  [kt]
  - r01: host-gathered [rand0 | rand1] pair                      [k3 ]
  - wh:  window half key, 64-row strip at partition half wh%2    [kt]
  - r2:  rand2 strip at the other half (host-gathered)           [k3 ]
  - G:   global pack {0,63} (l=1 / l=62 use 64-row edge strips)
wh/r2 share one 64-col PSUM region via tile_position partition halves;
wh's PV reads v directly out of the natural parity half of the vn
chunk, r2's PV uses a 2-up packed gather (odd l upper, even l lower).
Blocks l = 0, 63 attend densely to all keys.  QK -> one exp per group
(scale fused) -> PV with a ones-column on V accumulating the softmax
denominator in ctx row 64.  Host divides and transposes.

DMA: per-queue throughput is descriptor-gen-bound (~131 ns per SBUF
partition row), so inputs are merged into TWO wide tensors: k64 (all
64-row data: q^T, k^T, packs, key gathers; 40 KB rows) and v128 (all
128-row V data with ones columns).  Output is staged in SBUF and
shipped as bf16 ctx + f32 denominator row.
"""

import numpy as np

B, H, S, D = 2, 12, 4096, 64
BLK = 64
NB = S // BLK            # 64
NPAIR = B * H            # 24
NCORE = 8
PPC = NPAIR // NCORE     # 3 pairs per core
NMID = 62                # l = 1..62
NT = 31                  # r2 duo chunks
SCALE = 0.125            # 1/sqrt(64)

# kA column layout (dense-critical 64-row data)
K_KT = 0
K_KTG = S
K_QTD = S + 128
K_VGE = S + 256
KA_W = S + 256 + 130              # 4482
KG_W = NMID * 256                 # key gathers: per l, [r0|r1|wh|r2]
VA_W = 33 * 65                    # vne: 32 v-chunks + global pack
VB_W = 124 * 65                   # per l: [v_r0;v_r1|1][v_wh;v_r2|1]

_COMPILED = {}


def _build_host_arrays(query_layer, key_layer, value_layer, rand_attn):
    import ml_dtypes
    bf16 = ml_dtypes.bfloat16

    q = np.ascontiguousarray(query_layer, dtype=np.float32).reshape(NPAIR, S, D)
    k = np.ascontiguousarray(key_layer, dtype=np.float32).reshape(NPAIR, S, D)
    v = np.ascontiguousarray(value_layer, dtype=np.float32).reshape(NPAIR, S, D)
    r = np.ascontiguousarray(rand_attn, dtype=np.int64).reshape(NPAIR, NMID, 3)

    qt = q.transpose(0, 2, 1)                                # [24, 64, S]
    kt = k.transpose(0, 2, 1)

    kb = k.reshape(NPAIR, NB, BLK, D)
    vb = v.reshape(NPAIR, NB, BLK, D)
    bh = np.arange(NPAIR)[:, None, None]

    # ---- k64 ----
    ktg = np.concatenate([kb[:, 0], kb[:, NB - 1]], axis=1).transpose(0, 2, 1)
    qb = q.reshape(NPAIR, NB, BLK, D)
    qtd = np.concatenate([qb[:, 0], qb[:, NB - 1]], axis=1).transpose(0, 2, 1)
    one = np.ones((NPAIR, BLK, 1), np.float32)
    v63 = np.concatenate([vb[:, NB - 1], one], axis=2)       # [24, 64, 65]
    v0 = np.concatenate([vb[:, 0], one], axis=2)
    vge = np.concatenate([v63, v0], axis=2)                  # [24, 64, 130]
    ls_ = np.arange(1, NMID + 1)
    whb = np.where(ls_ % 2 == 1, ls_ + 1, ls_ - 1)
    whb = np.broadcast_to(whb[None, :], (NPAIR, NMID))
    i4 = np.concatenate([r[:, :, 0:2], whb[:, :, None], r[:, :, 2:3]], axis=2)
    gk = kb[bh, i4]                                          # [24, 62, 4, 64, 64]
    g3 = gk.transpose(0, 4, 1, 2, 3).reshape(NPAIR, D, NMID * 256)
    ka = np.ascontiguousarray(
        np.concatenate([kt, ktg, qtd, vge], axis=2)).astype(bf16)
    kq = np.ascontiguousarray(qt).astype(bf16)
    kg = np.ascontiguousarray(g3).astype(bf16)

    # ---- v128 ----
    vch = v.reshape(NPAIR, NB // 2, 128, D)
    o32 = np.ones((NPAIR, NB // 2, 128, 1), np.float32)
    vn = np.concatenate([vch, o32], axis=3)                  # [24, 32, 128, 65]
    gvg = np.concatenate([vb[:, 0], vb[:, NB - 1]], axis=1)  # [24, 128, 64]
    vg = np.concatenate([gvg, np.ones((NPAIR, 128, 1), np.float32)],
                        axis=2)[:, None]                     # [24, 1, 128, 65]
    vne = np.concatenate([vn, vg], axis=1)                   # [24, 33, 128, 65]

    gv01 = vb[bh, r[:, :, 0:2]].reshape(NPAIR, NMID, 128, D)
    o62 = np.ones((NPAIR, NMID, 128, 1), np.float32)
    vrr = np.concatenate([gv01, o62], axis=3)                # [24, 62, 128, 65]

    gvm = vb[bh, i4[:, :, 2:4]].reshape(NPAIR, NMID, 128, D)
    vmm = np.concatenate([gvm, o62], axis=3)                 # [24, 62, 128, 65]
    # interleave per l: [vrr_l | vmm_l] -> [24, 62, 2, 128, 65]
    vri = np.stack([vrr, vmm], axis=2).reshape(NPAIR, NMID * 2, 128, 65)

    va = np.ascontiguousarray(
        vne.transpose(0, 2, 1, 3).reshape(NPAIR, 128, VA_W)).astype(bf16)
    vbt = np.ascontiguousarray(
        vri.transpose(0, 2, 1, 3).reshape(NPAIR, 128, VB_W)).astype(bf16)

    return dict(ka=ka, kq=kq, kg=kg, va=va, vb=vbt)


def _fixup_multiwait(nc, mybir):
    """Split >1-sem-wait instructions (the Tile exit drain) into single-wait
    NoOps: this walrus build's CTRL codegen has one wait slot."""
    for fn in nc.m.functions:
        for bb in fn.blocks:
            insts = list(bb.instructions)
            out = []
            for inst in insts:
                si = inst.sync_info
                if si is not None and len(si.on_wait) > 1:
                    waits = list(si.on_wait)
                    for kk, w in enumerate(waits[:-1]):
                        nop = mybir.InstNoOp(
                            name=f"{inst.name}-wsplit{kk}",
                            opcode="NoOp",
                            engine=inst.engine,
                            sync_info=mybir.SyncInfo(on_wait=[w], on_update=[]),
                        )
                        out.append(nop)
                    si.on_wait = [waits[-1]]
                    inst.sync_info = si
                out.append(inst)
            bb.instructions = out


def _group_plan():
    """11 groups covering l=1..62.

    qk job: (dst_off, width, src, ctx_off, mrows, obase)
      out = st[obase:obase+mrows, off:off+w]
      src: ('kt',col,w) ('ktg',off,w) ('k3r',i) ('k3x',i) ('kts',block)
    pv job: (pt_off, width, src, ctx_off, krows, rbase)
      rhs = pt[rbase:rbase+krows, off:off+w]
      src: ('vn',chunk) ('vg',) ('vge',which) ('vr',i) ('vnh',block)
           ('vr2',t)
    """
    groups = []

    def build(ls_, singles, duos, g_edges):
        base_l = ls_[0]
        qk, pv = [], []
        off = 0
        g_ls = [l for l in ls_ if l not in g_edges]
        assert g_ls == list(range(g_ls[0], g_ls[0] + len(g_ls)))
        w = len(g_ls) * BLK
        qk.append((off, w, ('ktg', 0, 128), (g_ls[0] - base_l) * BLK, 128, 0))
        pv.append((off, w, ('vg',), (g_ls[0] - base_l) * BLK, 128, 0))
        off += w
        for l in g_edges:
            ko, vw = ((64, 0) if l == 1 else (0, 1))
            qk.append((off, 64, ('ktg', ko, 64), (l - base_l) * BLK, 64, 0))
            pv.append((off, 64, ('vge', vw), (l - base_l) * BLK, 64, 0))
            off += 64
        for l in singles:
            p = l // 2 if l % 2 == 0 else (l - 1) // 2
            qk.append((off, 64, ('kt', p * 128, 128), (l - base_l) * BLK,
                       128, 0))
            pv.append((off, 64, ('vn', p), (l - base_l) * BLK, 128, 0))
            off += 64
        for le in duos:
            p = le // 2
            qk.append((off, 128, ('kt', p * 128, 128), (le - base_l) * BLK,
                       128, 0))
            pv.append((off, 128, ('vn', p), (le - base_l) * BLK, 128, 0))
            off += 128
        for l in ls_:
            i = l - 1
            qk.append((off, 64, ('k3r', i), (l - base_l) * BLK, 128, 0))
            pv.append((off, 64, ('vr', i), (l - base_l) * BLK, 128, 0))
            off += 64
        for l in ls_:
            i = l - 1
            c = (l - base_l) * BLK
            qk.append((off, 64, ('k3m', i), c, 128, 0))
            pv.append((off, 64, ('vm', i), c, 128, 0))
            off += 64
        for o_, w_, _s, _c, _m, _b in qk:
            assert o_ // 512 == (o_ + w_ - 1) // 512, (o_, w_)
        assert off <= 1536
        return dict(ls=ls_, qk=qk, pv=pv, used=off)

    groups.append(build([1, 2, 3, 4, 5], singles=[1], duos=[2, 4],
                        g_edges=[1]))
    for kk in range(1, 10):
        a = 6 * kk
        groups.append(build(list(range(a, a + 6)), singles=[],
                            duos=[a, a + 2, a + 4], g_edges=[]))
    groups.append(build([60, 61, 62], singles=[62], duos=[60], g_edges=[62]))

    assert [l for g in groups for l in g['ls']] == list(range(1, 63))
    return groups


GROUPS = _group_plan()


def _build_program(apply_fixup=True):
    import sys
    if "/opt/trn_rl_repo" not in sys.path:
        sys.path.insert(0, "/opt/trn_rl_repo")
    import concourse.bass as bass
    import concourse.mybir as mybir
    from concourse.tile import TileContext

    f32 = mybir.dt.float32
    bf16 = mybir.dt.bfloat16
    EXP = mybir.ActivationFunctionType.Exp

    nc = bass.Bass("TRN2", target_bir_lowering=False, debug=False,
                   num_devices=NCORE)

    d_ka = nc.dram_tensor("ka", [PPC, D, KA_W], bf16,
                          kind="ExternalInput").ap()
    d_kq = nc.dram_tensor("kq", [PPC, D, S], bf16, kind="ExternalInput").ap()
    d_kg = nc.dram_tensor("kg", [PPC, D, KG_W], bf16,
                          kind="ExternalInput").ap()
    d_va = nc.dram_tensor("va", [PPC, 128, VA_W], bf16,
                          kind="ExternalInput").ap()
    d_vb = nc.dram_tensor("vb", [PPC, 128, VB_W], bf16,
                          kind="ExternalInput").ap()
    d_ctx = nc.dram_tensor("ctx", [PPC, 64, S], bf16,
                           kind="ExternalOutput").ap()
    d_den = nc.dram_tensor("den", [PPC, 1, S], f32, kind="ExternalOutput").ap()

    DW = [(0, 12), (12, 12), (24, 8)]   # dense waves

    with TileContext(nc) as tc:
        with tc.tile_pool(name="sb", bufs=2) as sb, \
             tc.tile_pool(name="ps", bufs=2, space="PSUM") as ps, \
             tc.tile_pool(name="ptp", bufs=4) as ptp, \
             tc.tile_pool(name="aux", bufs=2) as aux:

            for p in range(PPC):
                ka = sb.tile([D, KA_W], bf16, name=f"ka{p}", tag="ka")
                kq = sb.tile([D, S], bf16, name=f"kq{p}", tag="kq")
                kg = sb.tile([D, KG_W], bf16, name=f"kg{p}", tag="kg")
                va = sb.tile([128, VA_W], bf16, name=f"va{p}", tag="va")
                vb = sb.tile([128, VB_W], bf16, name=f"vb{p}", tag="vb")
                nc.sync.dma_start(out=ka, in_=d_ka[p])
                nc.scalar.dma_start(out=va, in_=d_va[p])
                nc.sync.dma_start(out=kq, in_=d_kq[p])
                nc.scalar.dma_start(out=vb, in_=d_vb[p])
                nc.sync.dma_start(out=kg, in_=d_kg[p])

                qt = kq
                kt = ka[:, K_KT:K_KT + S]

                def src_k(src):
                    kind = src[0]
                    if kind == 'kt':
                        return kt[:, src[1]:src[1] + src[2]]
                    if kind == 'ktg':
                        return ka[:, K_KTG + src[1]:K_KTG + src[1] + src[2]]
                    if kind == 'k3r':
                        a = src[1] * 256
                        return kg[:, a:a + 128]
                    if kind == 'k3m':
                        a = src[1] * 256 + 128
                        return kg[:, a:a + 128]
                    raise KeyError(src)

                def src_v(src):
                    kind = src[0]
                    if kind == 'vn':
                        return va[:, src[1] * 65:(src[1] + 1) * 65]
                    if kind == 'vg':
                        return va[:, 32 * 65:33 * 65]
                    if kind == 'vge':
                        return ka[:, K_VGE + src[1] * 65:
                                  K_VGE + (src[1] + 1) * 65]
                    if kind == 'vr':
                        a = src[1] * 130
                        return vb[:, a:a + 65]
                    if kind == 'vm':
                        a = src[1] * 130 + 65
                        return vb[:, a:a + 65]
                    raise KeyError(src)

                og = aux.tile([128, S], bf16, name=f"og{p}", tag="og")
                ogd = aux.tile([1, S], f32, name=f"ogd{p}", tag="ogd")

                # ---------------- dense blocks l = 0, 63 ----------------
                qtd = ka[:, K_QTD:K_QTD + 128]
                ctxd = ps.tile([128, 512], f32, name=f"ctxd{p}", tag="ctx",
                               bufs=2)
                for wv, (c0, nch) in enumerate(DW):
                    wd = nch * 128
                    std = ps.tile([128, 1536], f32, name=f"std{p}_{wv}",
                                  tag="st", bufs=2)
                    for cc in range(nch):
                        c = c0 + cc
                        nc.tensor.matmul(
                            std[:, cc * 128:(cc + 1) * 128],
                            lhsT=kt[:, c * 128:(c + 1) * 128],
                            rhs=qtd,
                            start=True, stop=True,
                        )
                    ptd = ptp.tile([128, 1536], bf16, name=f"ptd{p}_{wv}",
                                   tag="pt", bufs=4)
                    nc.scalar.activation(ptd[:, 0:wd], std[:, 0:wd], EXP,
                                         scale=SCALE)
                    for cc in range(nch):
                        c = c0 + cc
                        nc.tensor.matmul(
                            ctxd[0:65, 0:128],
                            lhsT=va[:, c * 65:(c + 1) * 65],
                            rhs=ptd[:, cc * 128:(cc + 1) * 128],
                            start=(c == 0), stop=(c == 31),
                        )
                nc.vector.tensor_copy(og[0:64, 0:BLK], ctxd[0:64, 0:BLK])
                nc.vector.tensor_copy(og[0:64, S - BLK:S], ctxd[0:64, BLK:128])
                nc.vector.tensor_copy(ogd[0:1, 0:BLK], ctxd[64:65, 0:BLK])
                nc.vector.tensor_copy(ogd[0:1, S - BLK:S], ctxd[64:65, BLK:128])

                # ---------------- middle groups ----------------
                for g, plan in enumerate(GROUPS):
                    ls = plan['ls']
                    base_l = ls[0]
                    W = len(ls) * BLK
                    used = plan['used']

                    st = ps.tile([128, 1536], f32, name=f"st{p}_{g}", tag="st",
                                 bufs=2)
                    for off, w, src, _c, mrows, obase in plan['qk']:
                        nc.tensor.matmul(
                            st[obase:obase + mrows, off:off + w],
                            lhsT=src_k(src),
                            rhs=qt[:, (base_l * BLK) + _c:
                                   (base_l * BLK) + _c + w],
                            start=True, stop=True,
                        )
                    pt = ptp.tile([128, 1536], bf16, name=f"pt{p}_{g}",
                                  tag="pt", bufs=4)
                    nc.scalar.activation(pt[:, 0:used], st[:, 0:used], EXP,
                                         scale=SCALE)

                    ctx = ps.tile([128, 512], f32, name=f"ctx{p}_{g}",
                                  tag="ctx", bufs=2)
                    pv = plan['pv']
                    for idx, (off, w, src, c, krows, rbase) in enumerate(pv):
                        nc.tensor.matmul(
                            ctx[0:65, c:c + w],
                            lhsT=src_v(src),
                            rhs=pt[rbase:rbase + krows, off:off + w],
                            start=(idx == 0), stop=(idx == len(pv) - 1),
                        )

                    nc.vector.tensor_copy(
                        og[0:64, base_l * BLK: base_l * BLK + W],
                        ctx[0:64, 0:W])
                    nc.vector.tensor_copy(
                        ogd[0:1, base_l * BLK: base_l * BLK + W],
                        ctx[64:65, 0:W])
                    if g % 2 == 1 or g == len(GROUPS) - 1:
                        # ship blocks [prev..ls[-1]]; g=1 also covers dense
                        # block 0 staged at cols 0:64
                        lo = 0 if g == 1 else GROUPS[g - 1]['ls'][0] * BLK
                        hi = (ls[-1] + 1) * BLK
                        nc.gpsimd.dma_start(out=d_ctx[p][:, lo:hi],
                                            in_=og[0:64, lo:hi])
                nc.gpsimd.dma_start(out=d_ctx[p][:, S - BLK:S],
                                    in_=og[0:64, S - BLK:S])
                nc.gpsimd.dma_start(out=d_den[p], in_=ogd)

    if apply_fixup:
        _fixup_multiwait(nc, mybir)
    return nc


def _get_program():
    if "nc" not in _COMPILED:
        _COMPILED["nc"] = _build_program()
    return _COMPILED["nc"]


def kernel(query_layer, key_layer, value_layer, band_mask, from_mask, to_mask,
           from_blocked_mask, to_blocked_mask, rand_attn):
    import sys
    if "/opt/trn_rl_repo" not in sys.path:
        sys.path.insert(0, "/opt/trn_rl_repo")
    from concourse.bass_utils import run_bass_kernel_spmd

    arrs = _build_host_arrays(query_layer, key_layer, value_layer, rand_attn)
    nc = _get_program()

    in_maps = []
    for c in range(NCORE):
        sl = slice(c * PPC, (c + 1) * PPC)
        in_maps.append({k: np.ascontiguousarray(v[sl]) for k, v in arrs.items()})

    res = run_bass_kernel_spmd(nc, in_maps, list(range(NCORE)))

    ctx = np.stack([res.results[c]["ctx"] for c in range(NCORE)])
    den = np.stack([res.results[c]["den"] for c in range(NCORE)])
    ctx = ctx.reshape(NPAIR, 64, S).astype(np.float64)
    den = den.reshape(NPAIR, 1, S).astype(np.float64)
    ctx = ctx / den                                                  # [24,64,S]
    ctx = ctx.transpose(0, 2, 1).reshape(B, H, S, D)                 # [B,H,S,D]
    out = ctx.transpose(0, 2, 1, 3).astype(np.float32)               # [B,S,H,D]
    return np.ascontiguousarray(out)


# revision 11
# speedup vs baseline: 1.1818x; 1.0800x over previous
"""BigBird-style block-sparse attention on 8 Trainium2 NeuronCores.

Problem: B=2, H=12, S=4096, D=64, BLK=64 (64 blocks), R=3 random blocks.
All mask inputs are ones (per the generator spec); rand_attn drives the
gather structure and is read host-side (the program itself is static).

Sharding: 24 (b,h) pairs -> 3 per core (data + head parallel).

Per-pair device algorithm, "ST" layout (keys on partitions, queries on
the free axis).  Middle query block l (1..62) attends, garbage-free:
  - W01: key pair {2p, 2p+1} shared by the query duo (2p, 2p+1)  [kt]
  - r01: host-gathered [rand0 | rand1] pair                      [k3 ]
  - wh:  window half key, 64-row strip at partition half wh%2    [kt]
  - r2:  rand2 strip at the other half (host-gathered)           [k3 ]
  - G:   global pack {0,63} (l=1 / l=62 use 64-row edge strips)
wh/r2 share one 64-col PSUM region via tile_position partition halves;
wh's PV reads v directly out of the natural parity half of the vn
chunk, r2's PV uses a 2-up packed gather (odd l upper, even l lower).
Blocks l = 0, 63 attend densely to all keys.  QK -> one exp per group
(scale fused) -> PV with a ones-column on V accumulating the softmax
denominator in ctx row 64.  Host divides and transposes.

DMA: per-queue throughput is descriptor-gen-bound (~131 ns per SBUF
partition row), so inputs are merged into TWO wide tensors: k64 (all
64-row data: q^T, k^T, packs, key gathers; 40 KB rows) and v128 (all
128-row V data with ones columns).  Output is staged in SBUF and
shipped as bf16 ctx + f32 denominator row.
"""

import numpy as np

B, H, S, D = 2, 12, 4096, 64
BLK = 64
NB = S // BLK            # 64
NPAIR = B * H            # 24
NCORE = 8
PPC = NPAIR // NCORE     # 3 pairs per core
NMID = 62                # l = 1..62
NT = 31                  # r2 duo chunks
SCALE = 0.125            # 1/sqrt(64)

# kA column layout (dense-critical 64-row data)
K_KT = 0
K_KTG = S
K_QTD = S + 128
K_VGE = S + 256
KA_W = S + 256 + 130              # 4482
KG_W = NMID * 256                 # key gathers: per l, [r0|r1|wh|r2]
VA_W = 33 * 65                    # vne: 32 v-chunks + global pack
VB_W = 124 * 65                   # per l: [v_r0;v_r1|1][v_wh;v_r2|1]
# progressive split of the gather streams (middle indices i = l-1)
GSPL = (11, 35)                   # chunk A: i<11, B: 11<=i<35, C: rest

_COMPILED = {}


def _build_host_arrays(query_layer, key_layer, value_layer, rand_attn):
    import ml_dtypes
    bf16 = ml_dtypes.bfloat16

    q = np.ascontiguousarray(query_layer, dtype=np.float32).reshape(NPAIR, S, D)
    k = np.ascontiguousarray(key_layer, dtype=np.float32).reshape(NPAIR, S, D)
    v = np.ascontiguousarray(value_layer, dtype=np.float32).reshape(NPAIR, S, D)
    r = np.ascontiguousarray(rand_attn, dtype=np.int64).reshape(NPAIR, NMID, 3)

    qt = q.transpose(0, 2, 1)                                # [24, 64, S]
    kt = k.transpose(0, 2, 1)

    kb = k.reshape(NPAIR, NB, BLK, D)
    vb = v.reshape(NPAIR, NB, BLK, D)
    bh = np.arange(NPAIR)[:, None, None]

    # ---- k64 ----
    ktg = np.concatenate([kb[:, 0], kb[:, NB - 1]], axis=1).transpose(0, 2, 1)
    qb = q.reshape(NPAIR, NB, BLK, D)
    qtd = np.concatenate([qb[:, 0], qb[:, NB - 1]], axis=1).transpose(0, 2, 1)
    one = np.ones((NPAIR, BLK, 1), np.float32)
    v63 = np.concatenate([vb[:, NB - 1], one], axis=2)       # [24, 64, 65]
    v0 = np.concatenate([vb[:, 0], one], axis=2)
    vge = np.concatenate([v63, v0], axis=2)                  # [24, 64, 130]
    ls_ = np.arange(1, NMID + 1)
    whb = np.where(ls_ % 2 == 1, ls_ + 1, ls_ - 1)
    whb = np.broadcast_to(whb[None, :], (NPAIR, NMID))
    i4 = np.concatenate([r[:, :, 0:2], whb[:, :, None], r[:, :, 2:3]], axis=2)
    gk = kb[bh, i4]                                          # [24, 62, 4, 64, 64]
    g3 = gk.transpose(0, 4, 1, 2, 3).reshape(NPAIR, D, NMID * 256)
    ka = np.ascontiguousarray(
        np.concatenate([kt, ktg, qtd, vge], axis=2)).astype(bf16)
    kq = np.ascontiguousarray(qt).astype(bf16)
    a, b = GSPL
    kg1 = np.ascontiguousarray(g3[:, :, :a * 256]).astype(bf16)
    kg2 = np.ascontiguousarray(g3[:, :, a * 256:b * 256]).astype(bf16)
    kg3 = np.ascontiguousarray(g3[:, :, b * 256:]).astype(bf16)

    # ---- v128 ----
    vch = v.reshape(NPAIR, NB // 2, 128, D)
    o32 = np.ones((NPAIR, NB // 2, 128, 1), np.float32)
    vn = np.concatenate([vch, o32], axis=3)                  # [24, 32, 128, 65]
    gvg = np.concatenate([vb[:, 0], vb[:, NB - 1]], axis=1)  # [24, 128, 64]
    vg = np.concatenate([gvg, np.ones((NPAIR, 128, 1), np.float32)],
                        axis=2)[:, None]                     # [24, 1, 128, 65]
    vne = np.concatenate([vn, vg], axis=1)                   # [24, 33, 128, 65]

    gv01 = vb[bh, r[:, :, 0:2]].reshape(NPAIR, NMID, 128, D)
    o62 = np.ones((NPAIR, NMID, 128, 1), np.float32)
    vrr = np.concatenate([gv01, o62], axis=3)                # [24, 62, 128, 65]

    gvm = vb[bh, i4[:, :, 2:4]].reshape(NPAIR, NMID, 128, D)
    vmm = np.concatenate([gvm, o62], axis=3)                 # [24, 62, 128, 65]
    # interleave per l: [vrr_l | vmm_l] -> [24, 62, 2, 128, 65]
    vri = np.stack([vrr, vmm], axis=2).reshape(NPAIR, NMID * 2, 128, 65)

    va = np.ascontiguousarray(
        vne.transpose(0, 2, 1, 3).reshape(NPAIR, 128, VA_W)).astype(bf16)
    vbt = np.ascontiguousarray(
        vri.transpose(0, 2, 1, 3).reshape(NPAIR, 128, VB_W)).astype(bf16)

    return dict(ka=ka, kq=kq, kg1=kg1, kg2=kg2, kg3=kg3, va=va,
                vb1=np.ascontiguousarray(vbt[:, :, :a * 130]),
                vb2=np.ascontiguousarray(vbt[:, :, a * 130:b * 130]),
                vb3=np.ascontiguousarray(vbt[:, :, b * 130:]))


def _fixup_multiwait(nc, mybir):
    """Split >1-sem-wait instructions (the Tile exit drain) into single-wait
    NoOps: this walrus build's CTRL codegen has one wait slot."""
    for fn in nc.m.functions:
        for bb in fn.blocks:
            insts = list(bb.instructions)
            out = []
            for inst in insts:
                si = inst.sync_info
                if si is not None and len(si.on_wait) > 1:
                    waits = list(si.on_wait)
                    for kk, w in enumerate(waits[:-1]):
                        nop = mybir.InstNoOp(
                            name=f"{inst.name}-wsplit{kk}",
                            opcode="NoOp",
                            engine=inst.engine,
                            sync_info=mybir.SyncInfo(on_wait=[w], on_update=[]),
                        )
                        out.append(nop)
                    si.on_wait = [waits[-1]]
                    inst.sync_info = si
                out.append(inst)
            bb.instructions = out


def _group_plan():
    """11 groups covering l=1..62.

    qk job: (dst_off, width, src, ctx_off, mrows, obase)
      out = st[obase:obase+mrows, off:off+w]
      src: ('kt',col,w) ('ktg',off,w) ('k3r',i) ('k3x',i) ('kts',block)
    pv job: (pt_off, width, src, ctx_off, krows, rbase)
      rhs = pt[rbase:rbase+krows, off:off+w]
      src: ('vn',chunk) ('vg',) ('vge',which) ('vr',i) ('vnh',block)
           ('vr2',t)
    """
    groups = []

    def build(ls_, singles, duos, g_edges):
        base_l = ls_[0]
        qk, pv = [], []
        off = 0
        g_ls = [l for l in ls_ if l not in g_edges]
        assert g_ls == list(range(g_ls[0], g_ls[0] + len(g_ls)))
        w = len(g_ls) * BLK
        qk.append((off, w, ('ktg', 0, 128), (g_ls[0] - base_l) * BLK, 128, 0))
        pv.append((off, w, ('vg',), (g_ls[0] - base_l) * BLK, 128, 0))
        off += w
        for l in g_edges:
            ko, vw = ((64, 0) if l == 1 else (0, 1))
            qk.append((off, 64, ('ktg', ko, 64), (l - base_l) * BLK, 64, 0))
            pv.append((off, 64, ('vge', vw), (l - base_l) * BLK, 64, 0))
            off += 64
        for l in singles:
            p = l // 2 if l % 2 == 0 else (l - 1) // 2
            qk.append((off, 64, ('kt', p * 128, 128), (l - base_l) * BLK,
                       128, 0))
            pv.append((off, 64, ('vn', p), (l - base_l) * BLK, 128, 0))
            off += 64
        for le in duos:
            p = le // 2
            qk.append((off, 128, ('kt', p * 128, 128), (le - base_l) * BLK,
                       128, 0))
            pv.append((off, 128, ('vn', p), (le - base_l) * BLK, 128, 0))
            off += 128
        for l in ls_:
            i = l - 1
            qk.append((off, 64, ('k3r', i), (l - base_l) * BLK, 128, 0))
            pv.append((off, 64, ('vr', i), (l - base_l) * BLK, 128, 0))
            off += 64
        for l in ls_:
            i = l - 1
            c = (l - base_l) * BLK
            qk.append((off, 64, ('k3m', i), c, 128, 0))
            pv.append((off, 64, ('vm', i), c, 128, 0))
            off += 64
        for o_, w_, _s, _c, _m, _b in qk:
            assert o_ // 512 == (o_ + w_ - 1) // 512, (o_, w_)
        assert off <= 1536
        return dict(ls=ls_, qk=qk, pv=pv, used=off)

    groups.append(build([1, 2, 3, 4, 5], singles=[1], duos=[2, 4],
                        g_edges=[1]))
    for kk in range(1, 10):
        a = 6 * kk
        groups.append(build(list(range(a, a + 6)), singles=[],
                            duos=[a, a + 2, a + 4], g_edges=[]))
    groups.append(build([60, 61, 62], singles=[62], duos=[60], g_edges=[62]))

    assert [l for g in groups for l in g['ls']] == list(range(1, 63))
    return groups


GROUPS = _group_plan()


def _build_program(apply_fixup=True):
    import sys
    if "/opt/trn_rl_repo" not in sys.path:
        sys.path.insert(0, "/opt/trn_rl_repo")
    import concourse.bass as bass
    import concourse.mybir as mybir
    from concourse.tile import TileContext

    f32 = mybir.dt.float32
    bf16 = mybir.dt.bfloat16
    EXP = mybir.ActivationFunctionType.Exp

    nc = bass.Bass("TRN2", target_bir_lowering=False, debug=False,
                   num_devices=NCORE)

    d_ka = nc.dram_tensor("ka", [PPC, D, KA_W], bf16,
                          kind="ExternalInput").ap()
    d_kq = nc.dram_tensor("kq", [PPC, D, S], bf16, kind="ExternalInput").ap()
    ga, gb = GSPL
    d_kg = [nc.dram_tensor(f"kg{j+1}", [PPC, D, n * 256], bf16,
                           kind="ExternalInput").ap()
            for j, n in enumerate((ga, gb - ga, NMID - gb))]
    d_va = nc.dram_tensor("va", [PPC, 128, VA_W], bf16,
                          kind="ExternalInput").ap()
    d_vb = [nc.dram_tensor(f"vb{j+1}", [PPC, 128, n * 130], bf16,
                           kind="ExternalInput").ap()
            for j, n in enumerate((ga, gb - ga, NMID - gb))]
    d_ctx = nc.dram_tensor("ctx", [PPC, 64, S], bf16,
                           kind="ExternalOutput").ap()
    d_den = nc.dram_tensor("den", [PPC, 1, S], f32, kind="ExternalOutput").ap()

    DW = [(0, 12), (12, 12), (24, 8)]   # dense waves

    with TileContext(nc) as tc:
        with tc.tile_pool(name="sb", bufs=2) as sb, \
             tc.tile_pool(name="ps", bufs=2, space="PSUM") as ps, \
             tc.tile_pool(name="ptp", bufs=4) as ptp, \
             tc.tile_pool(name="aux", bufs=2) as aux:

            for p in range(PPC):
                ka = sb.tile([D, KA_W], bf16, name=f"ka{p}", tag="ka")
                kq = sb.tile([D, S], bf16, name=f"kq{p}", tag="kq")
                va = sb.tile([128, VA_W], bf16, name=f"va{p}", tag="va")
                kgs = [sb.tile([D, n * 256], bf16, name=f"kg{j}{p}",
                               tag=f"kg{j}")
                       for j, n in enumerate((ga, gb - ga, NMID - gb))]
                vbs = [sb.tile([128, n * 130], bf16, name=f"vb{j}{p}",
                               tag=f"vb{j}")
                       for j, n in enumerate((ga, gb - ga, NMID - gb))]
                nc.sync.dma_start(out=ka, in_=d_ka[p])
                nc.scalar.dma_start(out=va, in_=d_va[p])
                nc.sync.dma_start(out=kq, in_=d_kq[p])
                for j in range(3):
                    nc.sync.dma_start(out=kgs[j], in_=d_kg[j][p])
                    nc.scalar.dma_start(out=vbs[j], in_=d_vb[j][p])

                def gidx(i):
                    return (0, i) if i < ga else                            ((1, i - ga) if i < gb else (2, i - gb))

                qt = kq
                kt = ka[:, K_KT:K_KT + S]

                def src_k(src):
                    kind = src[0]
                    if kind == 'kt':
                        return kt[:, src[1]:src[1] + src[2]]
                    if kind == 'ktg':
                        return ka[:, K_KTG + src[1]:K_KTG + src[1] + src[2]]
                    if kind == 'k3r':
                        j, i = gidx(src[1])
                        return kgs[j][:, i * 256:i * 256 + 128]
                    if kind == 'k3m':
                        j, i = gidx(src[1])
                        return kgs[j][:, i * 256 + 128:(i + 1) * 256]
                    raise KeyError(src)

                def src_v(src):
                    kind = src[0]
                    if kind == 'vn':
                        return va[:, src[1] * 65:(src[1] + 1) * 65]
                    if kind == 'vg':
                        return va[:, 32 * 65:33 * 65]
                    if kind == 'vge':
                        return ka[:, K_VGE + src[1] * 65:
                                  K_VGE + (src[1] + 1) * 65]
                    if kind == 'vr':
                        j, i = gidx(src[1])
                        return vbs[j][:, i * 130:i * 130 + 65]
                    if kind == 'vm':
                        j, i = gidx(src[1])
                        return vbs[j][:, i * 130 + 65:(i + 1) * 130]
                    raise KeyError(src)

                og = aux.tile([128, S], bf16, name=f"og{p}", tag="og")
                ogd = aux.tile([1, S], f32, name=f"ogd{p}", tag="ogd")

                # ---------------- dense blocks l = 0, 63 ----------------
                qtd = ka[:, K_QTD:K_QTD + 128]
                ctxd = ps.tile([128, 512], f32, name=f"ctxd{p}", tag="ctx",
                               bufs=2)
                for wv, (c0, nch) in enumerate(DW):
                    wd = nch * 128
                    std = ps.tile([128, 1536], f32, name=f"std{p}_{wv}",
                                  tag="st", bufs=2)
                    for cc in range(nch):
                        c = c0 + cc
                        nc.tensor.matmul(
                            std[:, cc * 128:(cc + 1) * 128],
                            lhsT=kt[:, c * 128:(c + 1) * 128],
                            rhs=qtd,
                            start=True, stop=True,
                        )
                    ptd = ptp.tile([128, 1536], bf16, name=f"ptd{p}_{wv}",
                                   tag="pt", bufs=4)
                    nc.scalar.activation(ptd[:, 0:wd], std[:, 0:wd], EXP,
                                         scale=SCALE)
                    for cc in range(nch):
                        c = c0 + cc
                        nc.tensor.matmul(
                            ctxd[0:65, 0:128],
                            lhsT=va[:, c * 65:(c + 1) * 65],
                            rhs=ptd[:, cc * 128:(cc + 1) * 128],
                            start=(c == 0), stop=(c == 31),
                        )
                nc.vector.tensor_copy(og[0:64, 0:BLK], ctxd[0:64, 0:BLK])
                nc.vector.tensor_copy(og[0:64, S - BLK:S], ctxd[0:64, BLK:128])
                nc.vector.tensor_copy(ogd[0:1, 0:BLK], ctxd[64:65, 0:BLK])
                nc.vector.tensor_copy(ogd[0:1, S - BLK:S], ctxd[64:65, BLK:128])

                # ---------------- middle groups ----------------
                for g, plan in enumerate(GROUPS):
                    ls = plan['ls']
                    base_l = ls[0]
                    W = len(ls) * BLK
                    used = plan['used']

                    st = ps.tile([128, 1536], f32, name=f"st{p}_{g}", tag="st",
                                 bufs=2)
                    for off, w, src, _c, mrows, obase in plan['qk']:
                        nc.tensor.matmul(
                            st[obase:obase + mrows, off:off + w],
                            lhsT=src_k(src),
                            rhs=qt[:, (base_l * BLK) + _c:
                                   (base_l * BLK) + _c + w],
                            start=True, stop=True,
                        )
                    pt = ptp.tile([128, 1536], bf16, name=f"pt{p}_{g}",
                                  tag="pt", bufs=4)
                    nc.scalar.activation(pt[:, 0:used], st[:, 0:used], EXP,
                                         scale=SCALE)

                    ctx = ps.tile([128, 512], f32, name=f"ctx{p}_{g}",
                                  tag="ctx", bufs=2)
                    pv = plan['pv']
                    for idx, (off, w, src, c, krows, rbase) in enumerate(pv):
                        nc.tensor.matmul(
                            ctx[0:65, c:c + w],
                            lhsT=src_v(src),
                            rhs=pt[rbase:rbase + krows, off:off + w],
                            start=(idx == 0), stop=(idx == len(pv) - 1),
                        )

                    nc.vector.tensor_copy(
                        og[0:64, base_l * BLK: base_l * BLK + W],
                        ctx[0:64, 0:W])
                    nc.vector.tensor_copy(
                        ogd[0:1, base_l * BLK: base_l * BLK + W],
                        ctx[64:65, 0:W])
                    if g % 2 == 1 or g == len(GROUPS) - 1:
                        # ship blocks [prev..ls[-1]]; g=1 also covers dense
                        # block 0 staged at cols 0:64
                        lo = 0 if g == 1 else GROUPS[g - 1]['ls'][0] * BLK
                        hi = (ls[-1] + 1) * BLK
                        nc.gpsimd.dma_start(out=d_ctx[p][:, lo:hi],
                                            in_=og[0:64, lo:hi])
                        nc.gpsimd.dma_start(out=d_den[p][:, lo:hi],
                                            in_=ogd[:, lo:hi])
                nc.gpsimd.dma_start(out=d_ctx[p][:, S - BLK:S],
                                    in_=og[0:64, S - BLK:S])
                nc.gpsimd.dma_start(out=d_den[p][:, S - BLK:S],
                                    in_=ogd[:, S - BLK:S])

    if apply_fixup:
        _fixup_multiwait(nc, mybir)
    return nc


def _get_program():
    if "nc" not in _COMPILED:
        _COMPILED["nc"] = _build_program()
    return _COMPILED["nc"]


def kernel(query_layer, key_layer, value_layer, band_mask, from_mask, to_mask,
           from_blocked_mask, to_blocked_mask, rand_attn):
    import sys
    if "/opt/trn_rl_repo" not in sys.path:
        sys.path.insert(0, "/opt/trn_rl_repo")
    from concourse.bass_utils import run_bass_kernel_spmd

    arrs = _build_host_arrays(query_layer, key_layer, value_layer, rand_attn)
    nc = _get_program()

    in_maps = []
    for c in range(NCORE):
        sl = slice(c * PPC, (c + 1) * PPC)
        in_maps.append({k: np.ascontiguousarray(v[sl]) for k, v in arrs.items()})

    res = run_bass_kernel_spmd(nc, in_maps, list(range(NCORE)))

    ctx = np.stack([res.results[c]["ctx"] for c in range(NCORE)])
    den = np.stack([res.results[c]["den"] for c in range(NCORE)])
    ctx = ctx.reshape(NPAIR, 64, S).astype(np.float64)
    den = den.reshape(NPAIR, 1, S).astype(np.float64)
    ctx = ctx / den                                                  # [24,64,S]
    ctx = ctx.transpose(0, 2, 1).reshape(B, H, S, D)                 # [B,H,S,D]
    out = ctx.transpose(0, 2, 1, 3).astype(np.float32)               # [B,S,H,D]
    return np.ascontiguousarray(out)


# revision 12
# speedup vs baseline: 1.2866x; 1.0887x over previous
"""BigBird-style block-sparse attention on 8 Trainium2 NeuronCores.

Problem: B=2, H=12, S=4096, D=64, BLK=64 (64 blocks), R=3 random blocks.
All mask inputs are ones (per the generator spec); rand_attn drives the
gather structure and is read host-side (the program itself is static).

Sharding: 24 (b,h) pairs -> 3 per core (data + head parallel).

Per-pair device algorithm, "ST" layout (keys on partitions, queries on
the free axis).  Middle query block l (1..62) attends, garbage-free:
  - W01: key pair {2p, 2p+1} shared by the query duo (2p, 2p+1)  [kt]
  - r01: host-gathered [rand0 | rand1] pair                      [k3 ]
  - wh:  window half key, 64-row strip at partition half wh%2    [kt]
  - r2:  rand2 strip at the other half (host-gathered)           [k3 ]
  - G:   global pack {0,63} (l=1 / l=62 use 64-row edge strips)
wh/r2 share one 64-col PSUM region via tile_position partition halves;
wh's PV reads v directly out of the natural parity half of the vn
chunk, r2's PV uses a 2-up packed gather (odd l upper, even l lower).
Blocks l = 0, 63 attend densely to all keys.  QK -> one exp per group
(scale fused) -> PV with a ones-column on V accumulating the softmax
denominator in ctx row 64.  Host divides and transposes.

DMA: per-queue throughput is descriptor-gen-bound (~131 ns per SBUF
partition row), so inputs are merged into TWO wide tensors: k64 (all
64-row data: q^T, k^T, packs, key gathers; 40 KB rows) and v128 (all
128-row V data with ones columns).  Output is staged in SBUF and
shipped as bf16 ctx + f32 denominator row.
"""

import numpy as np

B, H, S, D = 2, 12, 4096, 64
BLK = 64
NB = S // BLK            # 64
NPAIR = B * H            # 24
NCORE = 8
PPC = NPAIR // NCORE     # 3 pairs per core
NMID = 62                # l = 1..62
NT = 31                  # r2 duo chunks
SCALE = 0.125            # 1/sqrt(64)

# kA column layout (dense-critical 64-row data)
K_KT = 0
K_KTG = S
K_QTD = S + 128
K_VGE = S + 256
KA_W = S + 256 + 130              # 4482
KG_W = NMID * 256                 # key gathers: per l, [r0|r1|wh|r2]
VA_W = 33 * 65                    # vne: 32 v-chunks + global pack
VB_W = 124 * 65                   # per l: [v_r0;v_r1|1][v_wh;v_r2|1]
# progressive split of the gather streams (middle indices i = l-1)
GSPL = (11, 35)                   # chunk A: i<11, B: 11<=i<35, C: rest

_COMPILED = {}


def _build_host_arrays(query_layer, key_layer, value_layer, rand_attn):
    import ml_dtypes
    bf16 = ml_dtypes.bfloat16

    q = np.ascontiguousarray(query_layer, dtype=np.float32).reshape(NPAIR, S, D)
    k = np.ascontiguousarray(key_layer, dtype=np.float32).reshape(NPAIR, S, D)
    v = np.ascontiguousarray(value_layer, dtype=np.float32).reshape(NPAIR, S, D)
    r = np.ascontiguousarray(rand_attn, dtype=np.int64).reshape(NPAIR, NMID, 3)

    qt = q.transpose(0, 2, 1)                                # [24, 64, S]
    kt = k.transpose(0, 2, 1)

    kb = k.reshape(NPAIR, NB, BLK, D)
    vb = v.reshape(NPAIR, NB, BLK, D)
    bh = np.arange(NPAIR)[:, None, None]

    # ---- k64 ----
    ktg = np.concatenate([kb[:, 0], kb[:, NB - 1]], axis=1).transpose(0, 2, 1)
    qb = q.reshape(NPAIR, NB, BLK, D)
    qtd = np.concatenate([qb[:, 0], qb[:, NB - 1]], axis=1).transpose(0, 2, 1)
    one = np.ones((NPAIR, BLK, 1), np.float32)
    v63 = np.concatenate([vb[:, NB - 1], one], axis=2)       # [24, 64, 65]
    v0 = np.concatenate([vb[:, 0], one], axis=2)
    vge = np.concatenate([v63, v0], axis=2)                  # [24, 64, 130]
    ls_ = np.arange(1, NMID + 1)
    whb = np.where(ls_ % 2 == 1, ls_ + 1, ls_ - 1)
    whb = np.broadcast_to(whb[None, :], (NPAIR, NMID))
    i4 = np.concatenate([r[:, :, 0:2], whb[:, :, None], r[:, :, 2:3]], axis=2)
    gk = kb[bh, i4]                                          # [24, 62, 4, 64, 64]
    g3 = gk.transpose(0, 4, 1, 2, 3).reshape(NPAIR, D, NMID * 256)
    ka = np.ascontiguousarray(
        np.concatenate([kt, ktg, qtd, vge], axis=2)).astype(bf16)
    kq = np.ascontiguousarray(qt).astype(bf16)
    a, b = GSPL
    kg1 = np.ascontiguousarray(g3[:, :, :a * 256]).astype(bf16)
    kg2 = np.ascontiguousarray(g3[:, :, a * 256:b * 256]).astype(bf16)
    kg3 = np.ascontiguousarray(g3[:, :, b * 256:]).astype(bf16)

    # ---- v128 ----
    vch = v.reshape(NPAIR, NB // 2, 128, D)
    o32 = np.ones((NPAIR, NB // 2, 128, 1), np.float32)
    vn = np.concatenate([vch, o32], axis=3)                  # [24, 32, 128, 65]
    gvg = np.concatenate([vb[:, 0], vb[:, NB - 1]], axis=1)  # [24, 128, 64]
    vg = np.concatenate([gvg, np.ones((NPAIR, 128, 1), np.float32)],
                        axis=2)[:, None]                     # [24, 1, 128, 65]
    vne = np.concatenate([vn, vg], axis=1)                   # [24, 33, 128, 65]

    gv01 = vb[bh, r[:, :, 0:2]].reshape(NPAIR, NMID, 128, D)
    o62 = np.ones((NPAIR, NMID, 128, 1), np.float32)
    vrr = np.concatenate([gv01, o62], axis=3)                # [24, 62, 128, 65]

    gvm = vb[bh, i4[:, :, 2:4]].reshape(NPAIR, NMID, 128, D)
    vmm = np.concatenate([gvm, o62], axis=3)                 # [24, 62, 128, 65]
    # interleave per l: [vrr_l | vmm_l] -> [24, 62, 2, 128, 65]
    vri = np.stack([vrr, vmm], axis=2).reshape(NPAIR, NMID * 2, 128, 65)

    va = np.ascontiguousarray(
        vne.transpose(0, 2, 1, 3).reshape(NPAIR, 128, VA_W)).astype(bf16)
    vbt = np.ascontiguousarray(
        vri.transpose(0, 2, 1, 3).reshape(NPAIR, 128, VB_W)).astype(bf16)

    return dict(ka=ka, kq=kq, kg1=kg1, kg2=kg2, kg3=kg3, va=va,
                vb1=np.ascontiguousarray(vbt[:, :, :a * 130]),
                vb2=np.ascontiguousarray(vbt[:, :, a * 130:b * 130]),
                vb3=np.ascontiguousarray(vbt[:, :, b * 130:]))


def _fixup_multiwait(nc, mybir):
    """Split >1-sem-wait instructions (the Tile exit drain) into single-wait
    NoOps: this walrus build's CTRL codegen has one wait slot."""
    for fn in nc.m.functions:
        for bb in fn.blocks:
            insts = list(bb.instructions)
            out = []
            for inst in insts:
                si = inst.sync_info
                if si is not None and len(si.on_wait) > 1:
                    waits = list(si.on_wait)
                    for kk, w in enumerate(waits[:-1]):
                        nop = mybir.InstNoOp(
                            name=f"{inst.name}-wsplit{kk}",
                            opcode="NoOp",
                            engine=inst.engine,
                            sync_info=mybir.SyncInfo(on_wait=[w], on_update=[]),
                        )
                        out.append(nop)
                    si.on_wait = [waits[-1]]
                    inst.sync_info = si
                out.append(inst)
            bb.instructions = out


def _group_plan():
    """11 groups covering l=1..62.

    qk job: (dst_off, width, src, ctx_off, mrows, obase)
      out = st[obase:obase+mrows, off:off+w]
      src: ('kt',col,w) ('ktg',off,w) ('k3r',i) ('k3x',i) ('kts',block)
    pv job: (pt_off, width, src, ctx_off, krows, rbase)
      rhs = pt[rbase:rbase+krows, off:off+w]
      src: ('vn',chunk) ('vg',) ('vge',which) ('vr',i) ('vnh',block)
           ('vr2',t)
    """
    groups = []

    def build(ls_, singles, duos, g_edges):
        base_l = ls_[0]
        qk, pv = [], []
        off = 0
        g_ls = [l for l in ls_ if l not in g_edges]
        assert g_ls == list(range(g_ls[0], g_ls[0] + len(g_ls)))
        w = len(g_ls) * BLK
        qk.append((off, w, ('ktg', 0, 128), (g_ls[0] - base_l) * BLK, 128, 0))
        pv.append((off, w, ('vg',), (g_ls[0] - base_l) * BLK, 128, 0))
        off += w
        for l in g_edges:
            ko, vw = ((64, 0) if l == 1 else (0, 1))
            qk.append((off, 64, ('ktg', ko, 64), (l - base_l) * BLK, 64, 0))
            pv.append((off, 64, ('vge', vw), (l - base_l) * BLK, 64, 0))
            off += 64
        for l in singles:
            p = l // 2 if l % 2 == 0 else (l - 1) // 2
            qk.append((off, 64, ('kt', p * 128, 128), (l - base_l) * BLK,
                       128, 0))
            pv.append((off, 64, ('vn', p), (l - base_l) * BLK, 128, 0))
            off += 64
        for le in duos:
            p = le // 2
            qk.append((off, 128, ('kt', p * 128, 128), (le - base_l) * BLK,
                       128, 0))
            pv.append((off, 128, ('vn', p), (le - base_l) * BLK, 128, 0))
            off += 128
        for l in ls_:
            i = l - 1
            qk.append((off, 64, ('k3r', i), (l - base_l) * BLK, 128, 0))
            pv.append((off, 64, ('vr', i), (l - base_l) * BLK, 128, 0))
            off += 64
        for l in ls_:
            i = l - 1
            c = (l - base_l) * BLK
            qk.append((off, 64, ('k3m', i), c, 128, 0))
            pv.append((off, 64, ('vm', i), c, 128, 0))
            off += 64
        for o_, w_, _s, _c, _m, _b in qk:
            assert o_ // 512 == (o_ + w_ - 1) // 512, (o_, w_)
        assert off <= 1024
        return dict(ls=ls_, qk=qk, pv=pv, used=off)

    groups.append(build([1, 2, 3], singles=[1], duos=[2], g_edges=[1]))
    for kk in range(1, 15):
        a = 4 * kk
        groups.append(build(list(range(a, a + 4)), singles=[],
                            duos=[a, a + 2], g_edges=[]))
    groups.append(build([60, 61, 62], singles=[62], duos=[60], g_edges=[62]))

    assert [l for g in groups for l in g['ls']] == list(range(1, 63))
    return groups


GROUPS = _group_plan()


def _build_program(apply_fixup=True):
    import sys
    if "/opt/trn_rl_repo" not in sys.path:
        sys.path.insert(0, "/opt/trn_rl_repo")
    import concourse.bass as bass
    import concourse.mybir as mybir
    from concourse.tile import TileContext

    f32 = mybir.dt.float32
    bf16 = mybir.dt.bfloat16
    EXP = mybir.ActivationFunctionType.Exp

    nc = bass.Bass("TRN2", target_bir_lowering=False, debug=False,
                   num_devices=NCORE)

    d_ka = nc.dram_tensor("ka", [PPC, D, KA_W], bf16,
                          kind="ExternalInput").ap()
    d_kq = nc.dram_tensor("kq", [PPC, D, S], bf16, kind="ExternalInput").ap()
    ga, gb = GSPL
    d_kg = [nc.dram_tensor(f"kg{j+1}", [PPC, D, n * 256], bf16,
                           kind="ExternalInput").ap()
            for j, n in enumerate((ga, gb - ga, NMID - gb))]
    d_va = nc.dram_tensor("va", [PPC, 128, VA_W], bf16,
                          kind="ExternalInput").ap()
    d_vb = [nc.dram_tensor(f"vb{j+1}", [PPC, 128, n * 130], bf16,
                           kind="ExternalInput").ap()
            for j, n in enumerate((ga, gb - ga, NMID - gb))]
    d_ctx = nc.dram_tensor("ctx", [PPC, 64, S], bf16,
                           kind="ExternalOutput").ap()
    d_den = nc.dram_tensor("den", [PPC, 1, S], f32, kind="ExternalOutput").ap()

    DW = [(0, 8), (8, 8), (16, 8), (24, 8)]   # dense waves

    with TileContext(nc) as tc:
        with tc.tile_pool(name="sb", bufs=2) as sb, \
             tc.tile_pool(name="ps", bufs=2, space="PSUM") as ps, \
             tc.tile_pool(name="ptp", bufs=4) as ptp, \
             tc.tile_pool(name="aux", bufs=2) as aux:

            for p in range(PPC):
                ka = sb.tile([D, KA_W], bf16, name=f"ka{p}", tag="ka")
                kq = sb.tile([D, S], bf16, name=f"kq{p}", tag="kq")
                va = sb.tile([128, VA_W], bf16, name=f"va{p}", tag="va")
                kgs = [sb.tile([D, n * 256], bf16, name=f"kg{j}{p}",
                               tag=f"kg{j}")
                       for j, n in enumerate((ga, gb - ga, NMID - gb))]
                vbs = [sb.tile([128, n * 130], bf16, name=f"vb{j}{p}",
                               tag=f"vb{j}")
                       for j, n in enumerate((ga, gb - ga, NMID - gb))]
                nc.sync.dma_start(out=ka, in_=d_ka[p])
                nc.scalar.dma_start(out=va, in_=d_va[p])
                nc.sync.dma_start(out=kq, in_=d_kq[p])
                for j in range(3):
                    nc.sync.dma_start(out=kgs[j], in_=d_kg[j][p])
                    nc.scalar.dma_start(out=vbs[j], in_=d_vb[j][p])

                def gidx(i):
                    return (0, i) if i < ga else                            ((1, i - ga) if i < gb else (2, i - gb))

                qt = kq
                kt = ka[:, K_KT:K_KT + S]

                def src_k(src):
                    kind = src[0]
                    if kind == 'kt':
                        return kt[:, src[1]:src[1] + src[2]]
                    if kind == 'ktg':
                        return ka[:, K_KTG + src[1]:K_KTG + src[1] + src[2]]
                    if kind == 'k3r':
                        j, i = gidx(src[1])
                        return kgs[j][:, i * 256:i * 256 + 128]
                    if kind == 'k3m':
                        j, i = gidx(src[1])
                        return kgs[j][:, i * 256 + 128:(i + 1) * 256]
                    raise KeyError(src)

                def src_v(src):
                    kind = src[0]
                    if kind == 'vn':
                        return va[:, src[1] * 65:(src[1] + 1) * 65]
                    if kind == 'vg':
                        return va[:, 32 * 65:33 * 65]
                    if kind == 'vge':
                        return ka[:, K_VGE + src[1] * 65:
                                  K_VGE + (src[1] + 1) * 65]
                    if kind == 'vr':
                        j, i = gidx(src[1])
                        return vbs[j][:, i * 130:i * 130 + 65]
                    if kind == 'vm':
                        j, i = gidx(src[1])
                        return vbs[j][:, i * 130 + 65:(i + 1) * 130]
                    raise KeyError(src)

                og = aux.tile([128, S], bf16, name=f"og{p}", tag="og")
                ogd = aux.tile([1, S], f32, name=f"ogd{p}", tag="ogd")

                # ---------------- dense blocks l = 0, 63 ----------------
                qtd = ka[:, K_QTD:K_QTD + 128]
                ctxd = ps.tile([128, 512], f32, name=f"ctxd{p}", tag="ctx",
                               bufs=2)
                for wv, (c0, nch) in enumerate(DW):
                    wd = nch * 128
                    std = ps.tile([128, 1024], f32, name=f"std{p}_{wv}",
                                  tag="st", bufs=3)
                    for cc in range(nch):
                        c = c0 + cc
                        nc.tensor.matmul(
                            std[:, cc * 128:(cc + 1) * 128],
                            lhsT=kt[:, c * 128:(c + 1) * 128],
                            rhs=qtd,
                            start=True, stop=True,
                        )
                    ptd = ptp.tile([128, 1024], bf16, name=f"ptd{p}_{wv}",
                                   tag="pt", bufs=5)
                    nc.scalar.activation(ptd[:, 0:wd], std[:, 0:wd], EXP,
                                         scale=SCALE)
                    for cc in range(nch):
                        c = c0 + cc
                        nc.tensor.matmul(
                            ctxd[0:65, 0:128],
                            lhsT=va[:, c * 65:(c + 1) * 65],
                            rhs=ptd[:, cc * 128:(cc + 1) * 128],
                            start=(c == 0), stop=(c == 31),
                        )
                nc.vector.tensor_copy(og[0:64, 0:BLK], ctxd[0:64, 0:BLK])
                nc.vector.tensor_copy(og[0:64, S - BLK:S], ctxd[0:64, BLK:128])
                nc.vector.tensor_copy(ogd[0:1, 0:BLK], ctxd[64:65, 0:BLK])
                nc.vector.tensor_copy(ogd[0:1, S - BLK:S], ctxd[64:65, BLK:128])

                # ---------------- middle groups ----------------
                for g, plan in enumerate(GROUPS):
                    ls = plan['ls']
                    base_l = ls[0]
                    W = len(ls) * BLK
                    used = plan['used']

                    st = ps.tile([128, 1024], f32, name=f"st{p}_{g}", tag="st",
                                 bufs=3)
                    for off, w, src, _c, mrows, obase in plan['qk']:
                        nc.tensor.matmul(
                            st[obase:obase + mrows, off:off + w],
                            lhsT=src_k(src),
                            rhs=qt[:, (base_l * BLK) + _c:
                                   (base_l * BLK) + _c + w],
                            start=True, stop=True,
                        )
                    pt = ptp.tile([128, 1024], bf16, name=f"pt{p}_{g}",
                                  tag="pt", bufs=5)
                    nc.scalar.activation(pt[:, 0:used], st[:, 0:used], EXP,
                                         scale=SCALE)

                    ctx = ps.tile([128, 512], f32, name=f"ctx{p}_{g}",
                                  tag="ctx", bufs=2)
                    pv = plan['pv']
                    for idx, (off, w, src, c, krows, rbase) in enumerate(pv):
                        nc.tensor.matmul(
                            ctx[0:65, c:c + w],
                            lhsT=src_v(src),
                            rhs=pt[rbase:rbase + krows, off:off + w],
                            start=(idx == 0), stop=(idx == len(pv) - 1),
                        )

                    nc.vector.tensor_copy(
                        og[0:64, base_l * BLK: base_l * BLK + W],
                        ctx[0:64, 0:W])
                    nc.vector.tensor_copy(
                        ogd[0:1, base_l * BLK: base_l * BLK + W],
                        ctx[64:65, 0:W])
                    if g % 3 == 2 or g == len(GROUPS) - 1:
                        # ship blocks [prev..ls[-1]]; the first flush also
                        # covers dense block 0 staged at cols 0:64
                        lo = 0 if g <= 2 else GROUPS[g - 2]['ls'][0] * BLK
                        hi = (ls[-1] + 1) * BLK
                        nc.gpsimd.dma_start(out=d_ctx[p][:, lo:hi],
                                            in_=og[0:64, lo:hi])
                        nc.gpsimd.dma_start(out=d_den[p][:, lo:hi],
                                            in_=ogd[:, lo:hi])
                nc.gpsimd.dma_start(out=d_ctx[p][:, S - BLK:S],
                                    in_=og[0:64, S - BLK:S])
                nc.gpsimd.dma_start(out=d_den[p][:, S - BLK:S],
                                    in_=ogd[:, S - BLK:S])

    if apply_fixup:
        _fixup_multiwait(nc, mybir)
    return nc


def _get_program():
    if "nc" not in _COMPILED:
        _COMPILED["nc"] = _build_program()
    return _COMPILED["nc"]


def kernel(query_layer, key_layer, value_layer, band_mask, from_mask, to_mask,
           from_blocked_mask, to_blocked_mask, rand_attn):
    import sys
    if "/opt/trn_rl_repo" not in sys.path:
        sys.path.insert(0, "/opt/trn_rl_repo")
    from concourse.bass_utils import run_bass_kernel_spmd

    arrs = _build_host_arrays(query_layer, key_layer, value_layer, rand_attn)
    nc = _get_program()

    in_maps = []
    for c in range(NCORE):
        sl = slice(c * PPC, (c + 1) * PPC)
        in_maps.append({k: np.ascontiguousarray(v[sl]) for k, v in arrs.items()})

    res = run_bass_kernel_spmd(nc, in_maps, list(range(NCORE)))

    ctx = np.stack([res.results[c]["ctx"] for c in range(NCORE)])
    den = np.stack([res.results[c]["den"] for c in range(NCORE)])
    ctx = ctx.reshape(NPAIR, 64, S).astype(np.float64)
    den = den.reshape(NPAIR, 1, S).astype(np.float64)
    ctx = ctx / den                                                  # [24,64,S]
    ctx = ctx.transpose(0, 2, 1).reshape(B, H, S, D)                 # [B,H,S,D]
    out = ctx.transpose(0, 2, 1, 3).astype(np.float32)               # [B,S,H,D]
    return np.ascontiguousarray(out)
